# revision 30
# baseline (speedup 1.0000x reference)
"""Trainium2 Bass kernel for nn_ARTLearner: 2-layer tanh-RNN + MLP head.

Model (B=128, T=512, V=32000, E=512, H=2048):
    xs = emb[input_ids]                                    # [B, T, E]
    h0_t = tanh(x_t @ Wih0.T + bih0 + h0_{t-1} @ Whh0.T + bhh0)
    h1_t = tanh(h0_t @ Wih1.T + bih1 + h1_{t-1} @ Whh1.T + bhh1)
    out  = relu(h1_T @ W1.T + b1) @ W2.T + b2              # [B, V]

Strategy: tensor-parallel over H across 8 cores (each core owns a
256-wide slice of both hidden layers; batch B=128 occupies the full
PE-array M dim). The gathered hidden state is kept TRANSPOSED in SBUF
(hT: H on partitions, B on free) and serves as the matmul STATIONARY
operand; weight slices stream as rhs [128, 256], so one step is 52
matmuls instead of 104. Biases are folded in via a K=1 ones-row
matmul. Per step each core all-gathers its h0 slice (and, one step
lagged, its h1 slice) via two independent bf16 AllGather chains; the
chains interleave on the (serial) ncfw collective stream, which is the
measured bottleneck (~11.9us per 64KB-per-rank mesh AllGather + ~4us
inter-op gap -> ~32us/step). The embedding row gather (indirect DMA) +
PE-transpose to xT and the Wih0/Wih1 input transforms are fused into
the per-step matmul accumulation groups. Output slices are PE-transposed
back to hT form before each gather. Head is sharded over V (each core
computes [B, 4000] f32); the host concatenates along V.

All matmuls run in bf16 (fp32 PSUM accumulation); measured end-to-end
relative error vs the fp32 reference is 0.0089 (gate 2e-2), HW exec
16.50ms on 8 TRN2 cores.
"""

import contextlib
import ctypes
import os
import sys
import types

import numpy as np
import ml_dtypes

import concourse.bass as bass
import concourse.mybir as mybir
import concourse.tile as tile
from concourse import bacc
from concourse.masks import make_identity

# ─── axon NTFF profile hook shim (restores trace=True under this image) ──
def _install_ntff_hook():
    so_path = "/opt/axon/libaxon_pjrt.so"
    if "antenv.axon_hooks" not in sys.modules:
        mod = types.ModuleType("antenv.axon_hooks")
        holder = {"hook": None}
        mod.set_axon_ntff_profile_hook = lambda h: holder.__setitem__("hook", h)
        mod.get_axon_ntff_profile_hook = lambda: holder["hook"]
        sys.modules["antenv.axon_hooks"] = mod
        try:
            import antenv

            antenv.axon_hooks = mod
        except ImportError:
            pass
    m = sys.modules["antenv.axon_hooks"]
    if m.get_axon_ntff_profile_hook() is not None:
        return
    try:
        lib = ctypes.CDLL(so_path)
    except OSError:
        return
    if not hasattr(lib, "axon_start_nrt_profile"):
        return
    lib.axon_start_nrt_profile.argtypes = [ctypes.POINTER(ctypes.c_int64), ctypes.c_size_t]
    lib.axon_start_nrt_profile.restype = ctypes.c_int64
    lib.axon_stop_nrt_profile.argtypes = [ctypes.c_char_p]
    lib.axon_stop_nrt_profile.restype = ctypes.c_int64

    @contextlib.contextmanager
    def _hook(output_dir, device_ids):
        import jax

        jax.devices()
        if device_ids:
            ids = (ctypes.c_int64 * len(device_ids))(*device_ids)
            rc = lib.axon_start_nrt_profile(ids, len(device_ids))
        else:
            rc = lib.axon_start_nrt_profile(None, 0)
        if rc != 0:
            raise RuntimeError(f"axon_start_nrt_profile rc={rc}")
        try:
            yield
        finally:
            n = lib.axon_stop_nrt_profile(str(output_dir).encode())
            if n < 0:
                raise RuntimeError(f"axon_stop_nrt_profile rc={n}")

    m.set_axon_ntff_profile_hook(_hook)


# ─── problem constants (hardcoded per the spec) ──────────────────────────
B, T, V, E, H = 128, 512, 32000, 512, 2048
NC = 8                      # cores
HS = H // NC                # 256: per-core hidden slice
MC = HS // 128              # 2: m-chunks per slice
KH = H // 128               # 16: k-chunks over H
KE = E // 128               # 4:  k-chunks over E
VS = V // NC                # 4000: per-core vocab slice
BF16 = mybir.dt.bfloat16
F32 = mybir.dt.float32

# dummy matmuls per step to keep the PE HAM clock-gate warm across AllGather
# waits (idle >3.4us throttles the PE to 1.2GHz; measured 345ns/MM cold vs
# ~110ns warm). Each filler MM is [128,128]@[128,512] bf16 ~= 213ns warm.
FILLER_MM = int(os.environ.get("RNN_FILLER", "0"))
# one merged AllGather per step (both layers' slices) instead of two chains:
# collectives serialize on a single ncfw stream, so fewer+larger wins.
MERGED = os.environ.get("RNN_MERGED", "0") == "1"

last_exec_time_ns = None


def _build(t_steps=T):
    nc = bacc.Bacc("TRN2", target_bir_lowering=False, debug=False, num_devices=NC)
    d = {}
    d["emb"] = nc.dram_tensor("emb", [V, E], BF16, kind="ExternalInput").ap()
    d["ids"] = nc.dram_tensor("ids", [B, T], mybir.dt.int32, kind="ExternalInput").ap()
    d["whh0t"] = nc.dram_tensor("whh0t", [H, HS], BF16, kind="ExternalInput").ap()
    d["wih0t"] = nc.dram_tensor("wih0t", [E, HS], BF16, kind="ExternalInput").ap()
    d["wih1t"] = nc.dram_tensor("wih1t", [H, HS], BF16, kind="ExternalInput").ap()
    d["whh1t"] = nc.dram_tensor("whh1t", [H, HS], BF16, kind="ExternalInput").ap()
    d["bias0"] = nc.dram_tensor("bias0", [HS, 1], F32, kind="ExternalInput").ap()
    d["bias1"] = nc.dram_tensor("bias1", [HS, 1], F32, kind="ExternalInput").ap()
    d["bias0r"] = nc.dram_tensor("bias0r", [1, HS], BF16, kind="ExternalInput").ap()
    d["bias1r"] = nc.dram_tensor("bias1r", [1, HS], BF16, kind="ExternalInput").ap()
    d["w1t"] = nc.dram_tensor("w1t", [H, E], BF16, kind="ExternalInput").ap()
    d["b1"] = nc.dram_tensor("b1", [E, 1], F32, kind="ExternalInput").ap()
    d["w2t"] = nc.dram_tensor("w2t", [E, VS], BF16, kind="ExternalInput").ap()
    d["b2"] = nc.dram_tensor("b2", [1, VS], BF16, kind="ExternalInput").ap()
    d["out"] = nc.dram_tensor("out", [B, VS], F32, kind="ExternalOutput").ap()

    with tile.TileContext(nc) as tc:
        _body(nc, tc, d, t_steps)
    nc.compile()
    return nc


def _body(nc, tc, d, t_steps):
    Tanh = mybir.ActivationFunctionType.Tanh
    Relu = mybir.ActivationFunctionType.Relu
    ctx = contextlib.ExitStack()
    with ctx:
        wpool = ctx.enter_context(tc.tile_pool(name="weights", bufs=1))
        spool = ctx.enter_context(tc.tile_pool(name="state", bufs=2))
        xpool = ctx.enter_context(tc.tile_pool(name="xpipe", bufs=3))
        slpool = ctx.enter_context(tc.tile_pool(name="slices", bufs=2))
        pp = ctx.enter_context(tc.tile_pool(name="psum", bufs=2, space="PSUM"))
        ppx = ctx.enter_context(tc.tile_pool(name="psumx", bufs=3, space="PSUM"))
        ppf = ctx.enter_context(tc.tile_pool(name="psumf", bufs=1, space="PSUM"))
        dpool = ctx.enter_context(tc.tile_pool(name="dram", bufs=2, space="DRAM"))
        opool = ctx.enter_context(tc.tile_pool(name="outp", bufs=2))

        # ── prologue: weights into SBUF (resident) ──
        def load_w(name, ap, kchunks, free):
            t = wpool.tile([128, kchunks * free], BF16, tag=name)
            nc.sync.dma_start(
                t[:].rearrange("p (k f) -> p k f", k=kchunks),
                ap.rearrange("(k p) f -> p k f", p=128),
            )
            return t

        whh0 = load_w("whh0", d["whh0t"], KH, HS)   # [:, kc*HS + mc*128]
        wih0 = load_w("wih0", d["wih0t"], KE, HS)
        wih1 = load_w("wih1", d["wih1t"], KH, HS)
        whh1 = load_w("whh1", d["whh1t"], KH, HS)
        w1t = load_w("w1t", d["w1t"], KH, E)
        w2t = load_w("w2t", d["w2t"], KE, VS)

        ids_sb = wpool.tile([128, T], mybir.dt.int32, tag="ids")
        nc.sync.dma_start(ids_sb[:], d["ids"][:])
        bias0 = wpool.tile([128, MC], F32, tag="bias0")
        nc.sync.dma_start(
            bias0[:].rearrange("p (m o) -> p m o", m=MC),
            d["bias0"].rearrange("(m p) o -> p m o", p=128),
        )
        bias1 = wpool.tile([128, MC], F32, tag="bias1")
        nc.sync.dma_start(
            bias1[:].rearrange("p (m o) -> p m o", m=MC),
            d["bias1"].rearrange("(m p) o -> p m o", p=128),
        )
        b1sb = wpool.tile([128, KE], F32, tag="b1")
        nc.sync.dma_start(
            b1sb[:].rearrange("p (m o) -> p m o", m=KE),
            d["b1"].rearrange("(m p) o -> p m o", p=128),
        )
        b2sb = wpool.tile([1, VS], BF16, tag="b2")
        nc.sync.dma_start(b2sb[:], d["b2"][:])
        b0r = wpool.tile([1, HS], BF16, tag="b0r")
        nc.sync.dma_start(b0r[:], d["bias0r"][:])
        b1r = wpool.tile([1, HS], BF16, tag="b1r")
        nc.sync.dma_start(b1r[:], d["bias1r"][:])
        ones = wpool.tile([1, 128], BF16, tag="ones")
        nc.gpsimd.memset(ones[:], 1.0)
        ident = wpool.tile([128, 128], BF16, tag="ident")
        make_identity(nc, ident[:])

        # ── state buffers (gathered, transposed layout: [H-chunk part, B]) ──
        h0T = spool.tile([128, H], BF16, tag="h0T")
        nc.gpsimd.memset(h0T[:], 0.0)
        h1T = spool.tile([128, H], BF16, tag="h1T")
        nc.gpsimd.memset(h1T[:], 0.0)
        zsl = wpool.tile([128, HS], BF16, tag="zsl")
        nc.gpsimd.memset(zsl[:], 0.0)

        # shared DRAM landing zones for the two AllGather chains
        ccout0 = [
            nc.dram_tensor(f"ccout0_{s}", [H, B], BF16, kind="Internal", addr_space="Shared").ap()
            for s in range(2)
        ]
        ccout1 = [
            nc.dram_tensor(f"ccout1_{s}", [H, B], BF16, kind="Internal", addr_space="Shared").ap()
            for s in range(2)
        ]
        # merged-chain landing zones: both layers' slices in one AG
        ccoutm = [
            nc.dram_tensor(f"ccoutm_{s}", [2 * H, B], BF16, kind="Internal", addr_space="Shared").ap()
            for s in range(2)
        ]
        rg = [list(range(NC))]

        def make_xT(i):
            """Gather x(step i) = emb[ids[:, i-1]] and transpose to xT [E-part, B]."""
            xnat = xpool.tile([128, E], BF16, tag="xnat")
            nc.gpsimd.indirect_dma_start(
                out=xnat[:],
                out_offset=None,
                in_=d["emb"][:],
                in_offset=bass.IndirectOffsetOnAxis(ap=ids_sb[:, i - 1 : i], axis=0),
            )
            xps = ppx.tile([128, E], BF16, tag="xps")
            for c in range(KE):
                nc.tensor.transpose(
                    xps[:, c * 128 : (c + 1) * 128],
                    xnat[:, c * 128 : (c + 1) * 128],
                    ident[:],
                )
            xT = xpool.tile([128, E], BF16, tag="xT")
            nc.scalar.activation(xT[:], xps[:], mybir.ActivationFunctionType.Copy)
            return xT

        def gather(slice_sb, cc_tag, cc_out, state_tag):
            """AG one [128, HS] hT-form slice -> new full [128, H] state tile."""
            cc_in = dpool.tile([HS, B], BF16, tag=cc_tag)
            nc.sync.dma_start(
                cc_in[:].rearrange("(m p) b -> p m b", p=128),
                slice_sb[:].rearrange("p (m b) -> p m b", m=MC),
            )
            nc.gpsimd.collective_compute(
                "AllGather", mybir.AluOpType.bypass,
                replica_groups=rg, ins=[cc_in[:]], outs=[cc_out[:]],
            )
            new_state = spool.tile([128, H], BF16, tag=state_tag)
            nmap = new_state[:].rearrange("p (k b) -> p k b", k=KH)
            cmap = cc_out.rearrange("(k p) b -> p k b", p=128)
            for q in range(4):
                nc.sync.dma_start(
                    nmap[:, q * (KH // 4) : (q + 1) * (KH // 4), :],
                    cmap[:, q * (KH // 4) : (q + 1) * (KH // 4), :],
                )
            return new_state

        def gather_merged(h0sl_sb, h1sl_sb, cc_out):
            """One AG carrying both layers' hT slices.

            cc_in rows: [0:HS) = h0 slice, [HS:2*HS) = h1 slice. Gathered
            layout: rank r occupies rows [2*HS*r, 2*HS*(r+1)), first half h0.
            """
            cc_in = dpool.tile([2 * HS, B], BF16, tag="ccm")
            nc.sync.dma_start(
                cc_in[0:HS, :].rearrange("(m p) b -> p m b", p=128),
                h0sl_sb[:].rearrange("p (m b) -> p m b", m=MC),
            )
            nc.sync.dma_start(
                cc_in[HS : 2 * HS, :].rearrange("(m p) b -> p m b", p=128),
                h1sl_sb[:].rearrange("p (m b) -> p m b", m=MC),
            )
            nc.gpsimd.collective_compute(
                "AllGather", mybir.AluOpType.bypass,
                replica_groups=rg, ins=[cc_in[:]], outs=[cc_out[:]],
            )
            h0T_new = spool.tile([128, H], BF16, tag="h0T")
            h1T_new = spool.tile([128, H], BF16, tag="h1T")
            h0map = h0T_new[:].rearrange("p (r m b) -> p r m b", r=NC, m=MC)
            h1map = h1T_new[:].rearrange("p (r m b) -> p r m b", r=NC, m=MC)
            ccmap = cc_out.rearrange("(r s p) b -> p r s b", s=2 * MC, p=128)
            for mc in range(MC):
                nc.sync.dma_start(
                    h0map[:, :, mc : mc + 1, :], ccmap[:, :, mc : mc + 1, :]
                )
                nc.sync.dma_start(
                    h1map[:, :, mc : mc + 1, :], ccmap[:, :, MC + mc : MC + mc + 1, :]
                )
            return h0T_new, h1T_new

        # scratch operands for HAM-warmth filler matmuls
        fillw = wpool.tile([128, 128], BF16, tag="fillw")
        nc.gpsimd.memset(fillw[:], 0.0)
        fillr = wpool.tile([128, 512], BF16, tag="fillr")
        nc.gpsimd.memset(fillr[:], 0.0)

        def filler_block():
            if FILLER_MM <= 0:
                return
            fp = ppf.tile([128, 512], F32, tag="fill")
            for j in range(FILLER_MM):
                nc.tensor.matmul(fp[:], fillw[:], fillr[:], start=(j == 0), stop=(j == FILLER_MM - 1))

        xT = make_xT(1)
        def slice_to_hT(nat_sb, sl_tag):
            """[B, HS] natural slice -> [HS-part, B] hT-form via PE transpose."""
            tp = ppx.tile([128, HS], BF16, tag="xps")
            for mc in range(MC):
                nc.tensor.transpose(
                    tp[:, mc * 128 : (mc + 1) * 128],
                    nat_sb[:, mc * 128 : (mc + 1) * 128],
                    ident[:],
                )
            sl = slpool.tile([128, HS], BF16, tag=sl_tag)
            nc.scalar.activation(sl[:], tp[:], mybir.ActivationFunctionType.Copy)
            return sl

        for i in range(1, t_steps + 1):
            # ── mm_h0(i): h0 = tanh(x(i) @ Wih0.T + h0(i-1) @ Whh0.T + bias0)
            # activation-stationary orientation: out natural [B, HS], weights
            # stream as rhs [128, HS]. The x-part + bias go first — they only
            # need the prefetched xT, so they run inside the AG wait window.
            p0 = pp.tile([128, HS], F32, tag="p0")
            h0nat = slpool.tile([128, HS], BF16, tag="h0nat")
            for ec in range(KE):
                nc.tensor.matmul(
                    p0[:],
                    xT[:, ec * 128 : (ec + 1) * 128],
                    wih0[:, ec * HS : (ec + 1) * HS],
                    start=(ec == 0), stop=False,
                )
            nc.tensor.matmul(p0[:], ones[:], b0r[:], start=False, stop=False)
            for kc in range(KH):
                nc.tensor.matmul(
                    p0[:],
                    h0T[:, kc * 128 : (kc + 1) * 128],
                    whh0[:, kc * HS : (kc + 1) * HS],
                    start=False, stop=(kc == KH - 1),
                )
            nc.scalar.activation(h0nat[:], p0[:], Tanh)
            h0sl = slice_to_hT(h0nat, "h0sl")
            if not MERGED:
                # launch AG chain 0 for h0T(i)
                h0T_new = gather(h0sl, "cc0in", ccout0[i % 2], "h0T")

            # ── mm_h1(i-1): h1 slice = tanh(Wih1 @ h0T(i-1) + Whh1 @ h1T(i-2) + bias1)
            # (at i=1 the reference's h1(0) is the zero INITIAL state, not a
            #  computed step — keep the pre-zeroed buffer instead)
            if i >= 2:
                p1 = pp.tile([128, HS], F32, tag="p1")
                h1nat = slpool.tile([128, HS], BF16, tag="h1nat")
                nc.tensor.matmul(p1[:], ones[:], b1r[:], start=True, stop=False)
                for kc in range(KH):
                    nc.tensor.matmul(
                        p1[:],
                        h0T[:, kc * 128 : (kc + 1) * 128],
                        wih1[:, kc * HS : (kc + 1) * HS],
                        start=False, stop=False,
                    )
                for kc in range(KH):
                    nc.tensor.matmul(
                        p1[:],
                        h1T[:, kc * 128 : (kc + 1) * 128],
                        whh1[:, kc * HS : (kc + 1) * HS],
                        start=False, stop=(kc == KH - 1),
                    )
                nc.scalar.activation(h1nat[:], p1[:], Tanh)
                h1sl = slice_to_hT(h1nat, "h1sl")
                if not MERGED:
                    h1T_new = gather(h1sl, "cc1in", ccout1[i % 2], "h1T")
            else:
                h1sl = zsl
                if not MERGED:
                    h1T_new = h1T
            if MERGED:
                h0T_new, h1T_new = gather_merged(h0sl, h1sl, ccoutm[i % 2])

            # prefetch next step's xT while AGs fly
            if i < t_steps:
                xT = make_xT(i + 1)
            filler_block()
            h0T, h1T = h0T_new, h1T_new

        # ── epilogue: final h1 slice for step T, then gather full h1T(T) ──
        p1 = pp.tile([128, HS], F32, tag="p1")
        h1nat = slpool.tile([128, HS], BF16, tag="h1nat")
        nc.tensor.matmul(p1[:], ones[:], b1r[:], start=True, stop=False)
        for kc in range(KH):
            nc.tensor.matmul(
                p1[:],
                h0T[:, kc * 128 : (kc + 1) * 128],
                wih1[:, kc * HS : (kc + 1) * HS],
                start=False, stop=False,
            )
        for kc in range(KH):
            nc.tensor.matmul(
                p1[:],
                h1T[:, kc * 128 : (kc + 1) * 128],
                whh1[:, kc * HS : (kc + 1) * HS],
                start=False, stop=(kc == KH - 1),
            )
        nc.scalar.activation(h1nat[:], p1[:], Tanh)
        h1sl = slice_to_hT(h1nat, "h1sl")
        h1T = gather(h1sl, "cc1in", ccout1[(t_steps + 1) % 2], "h1T")

        # ── head: gT = relu(W1 @ h1T + b1) [E-part, B]; out = gT.T @ W2T + b2 ──
        gT = wpool.tile([128, E], BF16, tag="gT")
        for me in range(KE):
            pg = pp.tile([128, 128], F32, tag="p1")
            for kc in range(KH):
                nc.tensor.matmul(
                    pg[:],
                    w1t[:, kc * E + me * 128 : kc * E + (me + 1) * 128],
                    h1T[:, kc * 128 : (kc + 1) * 128],
                    start=(kc == 0), stop=(kc == KH - 1),
                )
            nc.scalar.activation(
                gT[:, me * 128 : (me + 1) * 128], pg[:], Relu, bias=b1sb[:, me : me + 1]
            )
        n_off = 0
        while n_off < VS:
            nsz = min(512, VS - n_off)
            pv = pp.tile([128, 512], F32, tag="p0")
            for ec in range(KE):
                nc.tensor.matmul(
                    pv[:, :nsz],
                    gT[:, ec * 128 : (ec + 1) * 128],
                    w2t[:, ec * VS + n_off : ec * VS + n_off + nsz],
                    start=(ec == 0), stop=False,
                )
            nc.tensor.matmul(
                pv[:, :nsz],
                ones[:],
                b2sb[:, n_off : n_off + nsz],
                start=False, stop=True,
            )
            osb = opool.tile([128, 512], F32, tag="osb")
            nc.vector.tensor_copy(osb[:, :nsz], pv[:, :nsz])
            nc.sync.dma_start(d["out"][:, n_off : n_off + nsz], osb[:, :nsz])
            n_off += nsz


_NC_CACHE = {}


def _get_nc(t_steps=T):
    if t_steps not in _NC_CACHE:
        _NC_CACHE[t_steps] = _build(t_steps)
    return _NC_CACHE[t_steps]


def _prep_in_maps(input_ids, emb, Wih0, Whh0, bih0, bhh0, Wih1, Whh1, bih1, bhh1, W1, b1, W2, b2):
    bf = lambda a: np.ascontiguousarray(np.asarray(a, dtype=np.float32)).astype(ml_dtypes.bfloat16)
    f32 = lambda a: np.ascontiguousarray(np.asarray(a, dtype=np.float32))
    ids = np.ascontiguousarray(np.asarray(input_ids).astype(np.int32))
    emb_bf = bf(emb)
    wih0t_f = f32(Wih0).T  # [E, H]
    whh0t_f = f32(Whh0).T  # [H, H]
    wih1t_f = f32(Wih1).T
    whh1t_f = f32(Whh1).T
    bias0_f = f32(bih0) + f32(bhh0)
    bias1_f = f32(bih1) + f32(bhh1)
    w1t_f = f32(W1).T      # [H, E]
    w2t_f = f32(W2).T      # [E, V]
    b1_f = f32(b1)
    b2_bf = bf(b2)

    in_maps = []
    for c in range(NC):
        sl = slice(c * HS, (c + 1) * HS)
        vsl = slice(c * VS, (c + 1) * VS)
        in_maps.append({
            "emb": emb_bf,
            "ids": ids,
            "whh0t": np.ascontiguousarray(whh0t_f[:, sl]).astype(ml_dtypes.bfloat16),
            "wih0t": np.ascontiguousarray(wih0t_f[:, sl]).astype(ml_dtypes.bfloat16),
            "wih1t": np.ascontiguousarray(wih1t_f[:, sl]).astype(ml_dtypes.bfloat16),
            "whh1t": np.ascontiguousarray(whh1t_f[:, sl]).astype(ml_dtypes.bfloat16),
            "bias0": np.ascontiguousarray(bias0_f[sl]).reshape(HS, 1),
            "bias1": np.ascontiguousarray(bias1_f[sl]).reshape(HS, 1),
            "bias0r": np.ascontiguousarray(bias0_f[sl]).reshape(1, HS).astype(ml_dtypes.bfloat16),
            "bias1r": np.ascontiguousarray(bias1_f[sl]).reshape(1, HS).astype(ml_dtypes.bfloat16),
            "w1t": np.ascontiguousarray(w1t_f).astype(ml_dtypes.bfloat16),
            "b1": np.ascontiguousarray(b1_f).reshape(E, 1),
            "w2t": np.ascontiguousarray(w2t_f[:, vsl]).astype(ml_dtypes.bfloat16),
            "b2": np.ascontiguousarray(b2_bf[vsl]).reshape(1, VS),
        })
    return in_maps


def kernel(**inputs):
    global last_exec_time_ns
    _install_ntff_hook()
    from concourse.bass_utils import run_bass_kernel_spmd

    nc = _get_nc()
    in_maps = _prep_in_maps(**inputs)
    try:
        res = run_bass_kernel_spmd(nc, in_maps, core_ids=list(range(NC)), trace=True)
    except Exception:
        res = run_bass_kernel_spmd(nc, in_maps, core_ids=list(range(NC)), trace=False)
    last_exec_time_ns = res.exec_time_ns
    if res.exec_time_ns is not None:
        print(f"HW exec time: {res.exec_time_ns} ns")
    out = np.concatenate([res.results[c]["out"] for c in range(NC)], axis=1)
    return out.astype(np.float32)
